# revision 10
# baseline (speedup 1.0000x reference)
"""LoRA layer kernel for Trainium2, 8-core data-parallel.

out = x @ W.T + 2.0 * ((x @ B) @ A)
  x: (4, 4096, 4096) f32, W: (4096, 4096), A: (16, 4096), B: (4096, 16)

Strategy: flatten x to (16384, 4096) rows, shard rows across 8 cores
(2048 rows each), replicate W/A/B. All matmul operands in bf16 (PSUM
accumulation stays fp32; end-to-end rel err ~2e-3 vs the 2e-2 gate).

Per core, single x-resident block (2048 rows = 128 KB/partition bf16):
  - x loads m-split into 512-col quarters so mt-quarter q + stage-A
    chunk h=q unlock after 1/4 of x lands (keeps PE fed during the
    HBM-bound load).
  - stage A (tT = (x @ B).T) 4-way column-tiled: k-tile k -> PE col
    group k%4, partials in 4 PSUM banks at partition stripes
    {32j..32j+15}; fused per-k with main quarter q=g for oc0 so the
    in-order PE queue never head-of-line blocks on x arrival.
  - LoRA fold for oc0: one K=128 matmul per mt against a2rep (2*A at
    the 4 stripes, zeros elsewhere; tT4 memset kills NaN garbage).
  - LoRA fold for oc1..7: partial stripes packed+duplicated into tTp
    rows 0-63 / 64-127 via SBUF->SBUF DMAs; accum is 2 concurrent
    row-tiled K=64 matmuls per quarter (tile_position (0,0)/(64,0)).
  - main GEMM: per o-chunk, 4 mt-quarters of 4 PSUM banks; x-tile
    stationary, W streamed as k-quads [128, 4*512] on sync.
Queues: W/consts/dups on sync, out stores alternate scalar/sync,
x loads alternate gpsimd/scalar.
"""

import sys

if "/opt/trn_rl_repo" not in sys.path:
    sys.path.insert(0, "/opt/trn_rl_repo")

import numpy as np
import ml_dtypes

import concourse.bass as bass
import concourse.mybir as mybir
import concourse.tile as tile

N_CORES = 8
D = 4096
RANK = 16
ROWS_TOTAL = 4 * 4096          # 16384
M = ROWS_TOTAL // N_CORES      # 2048 rows per core
P = 128
KT = D // P                    # 32 k-tiles
OC = 512                       # o-chunk width (one PSUM bank)
N_OC = D // OC                 # 8
MT = M // P                    # 16 m-tiles
MQ = 4                         # m-tiles per quarter (PSUM banks)
NQ = MT // MQ                  # 4 quarters
KQ = 4                         # k-tiles per W quad DMA
N_KQ = KT // KQ                # 8

F32 = mybir.dt.float32
BF16 = mybir.dt.bfloat16
BF16_NP = ml_dtypes.bfloat16

N_WARMUP = 8


def split_wide_waits(nc, max_waits=1):
    """walrus in this container rejects >1 sync wait per instruction;
    move excess waits onto preceding same-engine NoOps."""
    n_split = 0
    for f in nc.m.functions:
        for bb in f.blocks:
            new_insts = []
            for inst in bb.instructions:
                si = getattr(inst, "sync_info", None)
                if si is not None and si.on_wait and len(si.on_wait) > max_waits:
                    waits = list(si.on_wait)
                    keep = waits[-max_waits:]
                    extra = waits[:-max_waits]
                    for i in range(0, len(extra), max_waits):
                        chunk = extra[i:i + max_waits]
                        nop = mybir.InstNoOp(
                            name=f"{inst.name}_wsplit{i}",
                            sync_info=mybir.SyncInfo(on_wait=chunk, on_update=[]),
                            bass_nofuse=True,
                            engine=inst.engine,
                        )
                        new_insts.append(nop)
                        n_split += 1
                    si.on_wait = keep
                new_insts.append(inst)
            bb.instructions[:] = new_insts
    return n_split


def build_program():
    nc = bass.Bass()
    xt = nc.declare_dram_parameter("xt", [D, M], BF16, isOutput=False)
    wt = nc.declare_dram_parameter("wt", [D, D], BF16, isOutput=False)
    # bmat pre-arranged on host: [128, KT*RANK], col-block k = rows k*128..+128
    bmat = nc.declare_dram_parameter("bmat", [P, KT * RANK], BF16, isOutput=False)
    # a2rep: 2*A at partition stripes {32j..32j+15}, zeros elsewhere (K=128 fold)
    a2rep = nc.declare_dram_parameter("a2rep", [P, D], BF16, isOutput=False)
    # a2p: 2*A tiled 4x in rows 0-63 and again in rows 64-127 (K=64 pair fold)
    a2p = nc.declare_dram_parameter("a2p", [P, D], BF16, isOutput=False)
    out = nc.declare_dram_parameter("out", [M, D], F32, isOutput=True)

    with tile.TileContext(nc) as tc:
        with (
            tc.tile_pool(name="xpool", bufs=1) as xpool,
            tc.tile_pool(name="wpool", bufs=9) as wpool,
            tc.tile_pool(name="opool", bufs=2) as opool,
            tc.tile_pool(name="cpool", bufs=1) as cpool,
            tc.tile_pool(name="tpool", bufs=1) as tpool,
            tc.tile_pool(name="ppool", bufs=8, space="PSUM") as ppool,
        ):
            # constants on sync queue
            btile = cpool.tile([P, KT * RANK], BF16, tag="bt")
            nc.sync.dma_start(btile[:], bmat[:])
            atile_r = cpool.tile([P, D], BF16, tag="ar")
            nc.sync.dma_start(atile_r[:], a2rep[:])
            atile_p = cpool.tile([P, D], BF16, tag="ap")
            nc.sync.dma_start(atile_p[:], a2p[:])

            # x fully resident: col block k holds x.T[k*128:(k+1)*128, :]
            xall = xpool.tile([P, KT * M], BF16, tag="x")

            def xsl(k, c0, cw):
                return xall[:, k * M + c0: k * M + c0 + cw]

            # m-split x loads: m-quarter mq unlocks mt-quarter mq and
            # stage-A chunk h=mq after only 1/4 of x has landed.
            # mq0's tail (k24-31) goes on sync interleaved with W quads
            # (emitted in w_load(0)) so mq0 rides ~2.5 queues.
            def x_dma(eng, k, mq):
                eng.dma_start(
                    xsl(k, mq * OC, OC),
                    xt[k * P:(k + 1) * P, mq * OC:(mq + 1) * OC],
                )

            nx = 0
            for mq in range(NQ):
                for k in range(KT):
                    if mq == 0 and k >= 24:
                        continue
                    eng = nc.gpsimd if nx % 2 == 0 else nc.scalar
                    nx += 1
                    x_dma(eng, k, mq)

            # tT4 [128, M]: stage-A partials at stripes {32j..32j+15};
            # memset clears garbage stripes (read by the oc0 K=128 fold).
            tT4 = tpool.tile([P, M], BF16, tag="tT")
            nc.vector.memset(tT4[:], 0.0)
            # tTp [128, M]: stripes packed to rows {16j..16j+15} and
            # duplicated at rows 64+, for the row-tiled K=64 pair fold.
            tTp = tpool.tile([P, M], BF16, tag="tP")

            # static PSUM banks, allocated once and reused explicitly
            # (cuts pool-allocation semaphores and teardown cost)
            pbank = [
                ppool.tile([P, OC], F32, tag="acc", name=f"bank{i}")
                for i in range(8)
            ]

            # HAM warmup: dummy matmuls so the PE clock ramps to 8/8.
            junk = pbank[0]
            for i in range(N_WARMUP):
                nc.tensor.matmul(
                    junk[:],
                    btile[:, :P],
                    btile[:, :OC],
                    start=(i == 0),
                    stop=(i == N_WARMUP - 1),
                )

            def w_load(oc):
                wtiles = []
                for k4 in range(N_KQ):
                    wtile = wpool.tile([P, KQ * OC], BF16, tag="wt")
                    nc.sync.dma_start(
                        wtile.rearrange("p (b c) -> p b c", b=KQ),
                        wt[k4 * KQ * P:(k4 + 1) * KQ * P,
                           oc * OC:(oc + 1) * OC].rearrange(
                               "(b p) c -> p b c", p=P),
                    )
                    wtiles.append(wtile)
                    if oc == 0 and 3 <= k4 < 7:
                        # mq0 tail rides the sync queue between W quads
                        x_dma(nc.sync, 24 + 2 * (k4 - 3), 0)
                        x_dma(nc.sync, 25 + 2 * (k4 - 3), 0)
                return wtiles

            def finish_quarter(oc, q, psq, packed):
                ot = opool.tile([P, MQ * OC], F32, tag="ot")
                if packed:
                    # row-tiled K=64 pairs: even mi rows 0-63, odd 64-127
                    for mi in range(MQ):
                        mt = q * MQ + mi
                        r0 = 0 if mi % 2 == 0 else 64
                        nc.tensor.matmul(
                            psq[mi][:],
                            tTp[r0:r0 + 64, mt * P:(mt + 1) * P],
                            atile_p[r0:r0 + 64, oc * OC:(oc + 1) * OC],
                            start=False,
                            stop=True,
                            tile_position=(r0, 0),
                        )
                else:
                    for mi in range(MQ):
                        mt = q * MQ + mi
                        nc.tensor.matmul(
                            psq[mi][:],
                            tT4[:, mt * P:(mt + 1) * P],
                            atile_r[:, oc * OC:(oc + 1) * OC],
                            start=False,
                            stop=True,
                        )
                for mi in range(MQ):
                    nc.vector.tensor_copy(
                        ot[:, mi * OC:(mi + 1) * OC], psq[mi][:]
                    )
                seng = nc.scalar if (oc * NQ + q) % 2 == 0 else nc.sync
                seng.dma_start(
                    out[q * MQ * P:(q + 1) * MQ * P,
                        oc * OC:(oc + 1) * OC].rearrange(
                            "(b p) c -> p b c", p=P),
                    ot.rearrange("p (b c) -> p b c", b=MQ),
                )

            # oc0 fused with stage A: per k, 1 stage-A MM + 4 main MMs,
            # all gated on the same arriving x tile (k, mq=g).
            wtiles = w_load(0)
            for g in range(NQ):
                pa = pbank[0:4]
                psq = pbank[4:8]
                # k-quad interleave: 4 adjacent col-tiled stage-A MMs
                # (keeps their 4-way overlap), then 16 main MMs — all
                # gated on the same 4 arriving x tiles.
                for k4 in range(N_KQ):
                    for kk in range(KQ):
                        k = KQ * k4 + kk
                        j = k % 4
                        nc.tensor.matmul(
                            pa[j][32 * j:32 * j + RANK, :],
                            btile[:, k * RANK:(k + 1) * RANK],
                            xsl(k, g * OC, OC),
                            start=(k < 4),
                            stop=(k >= KT - 4),
                            tile_position=(0, 32 * j),
                        )
                    for kk in range(KQ):
                        k = KQ * k4 + kk
                        for mi in range(MQ):
                            mt = g * MQ + mi
                            nc.tensor.matmul(
                                psq[mi][:],
                                xsl(k, mt * P, P),
                                wtiles[k4][:, kk * OC:(kk + 1) * OC],
                                start=(k == 0),
                                stop=False,
                            )
                for j in range(4):
                    nc.vector.tensor_copy(
                        tT4[32 * j:32 * j + RANK, g * OC:(g + 1) * OC],
                        pa[j][32 * j:32 * j + RANK, :],
                    )
                # pack + duplicate stripes into tTp (SBUF->SBUF, sync queue)
                for j in range(4):
                    sstr = tT4[32 * j:32 * j + RANK, g * OC:(g + 1) * OC]
                    nc.gpsimd.dma_start(
                        tTp[16 * j:16 * j + RANK, g * OC:(g + 1) * OC], sstr)
                    nc.gpsimd.dma_start(
                        tTp[64 + 16 * j:64 + 16 * j + RANK,
                            g * OC:(g + 1) * OC], sstr)
                finish_quarter(0, g, psq, packed=False)

            for oc in range(1, N_OC):
                wtiles = w_load(oc)
                for q in range(NQ):
                    psq = pbank[0:4] if q % 2 == 0 else pbank[4:8]
                    for k4 in range(N_KQ):
                        for kk in range(KQ):
                            k = KQ * k4 + kk
                            for mi in range(MQ):
                                mt = q * MQ + mi
                                nc.tensor.matmul(
                                    psq[mi][:],
                                    xsl(k, mt * P, P),
                                    wtiles[k4][:, kk * OC:(kk + 1) * OC],
                                    start=(k == 0),
                                    stop=False,
                                )
                    finish_quarter(oc, q, psq, packed=True)

    split_wide_waits(nc)
    return nc


_NC_CACHE = [None]


def kernel(x, weight, lora_A, lora_B):
    from concourse.bass_utils import run_bass_kernel_spmd

    x = np.asarray(x, dtype=np.float32)
    weight = np.asarray(weight, dtype=np.float32)
    lora_A = np.asarray(lora_A, dtype=np.float32)
    lora_B = np.asarray(lora_B, dtype=np.float32)

    x2 = x.reshape(ROWS_TOTAL, D)
    wt = np.ascontiguousarray(weight.T).astype(BF16_NP)
    a2 = (2.0 * lora_A).astype(BF16_NP)
    # a2rep: 2*A at stripes {32j..32j+15}, zeros elsewhere
    a2rep = np.zeros((P, D), dtype=BF16_NP)
    for j in range(4):
        a2rep[32 * j:32 * j + RANK, :] = a2
    # a2p: 2*A tiled at rows {16j..16j+15} and duplicated at rows 64+
    a2p = np.zeros((P, D), dtype=BF16_NP)
    for j in range(4):
        a2p[16 * j:16 * j + RANK, :] = a2
        a2p[64 + 16 * j:64 + 16 * j + RANK, :] = a2
    # pre-arrange B: [128, KT*RANK], col-block k holds rows k*128..(k+1)*128
    bmat = np.ascontiguousarray(
        lora_B.reshape(KT, P, RANK).transpose(1, 0, 2).reshape(P, KT * RANK)
    ).astype(BF16_NP)

    in_maps = []
    for c in range(N_CORES):
        xt_c = np.ascontiguousarray(
            x2[c * M:(c + 1) * M].T
        ).astype(BF16_NP)
        in_maps.append({"xt": xt_c, "wt": wt, "bmat": bmat,
                        "a2rep": a2rep, "a2p": a2p})

    if _NC_CACHE[0] is None:
        _NC_CACHE[0] = build_program()
    nc = _NC_CACHE[0]

    res = run_bass_kernel_spmd(nc, in_maps, list(range(N_CORES)))
    out = np.concatenate(
        [res.results[c]["out"] for c in range(N_CORES)], axis=0
    )
    return out.reshape(x.shape)


# revision 11
# speedup vs baseline: 1.0036x; 1.0036x over previous
"""LoRA layer kernel for Trainium2, 8-core data-parallel.

out = x @ W.T + 2.0 * ((x @ B) @ A)
  x: (4, 4096, 4096) f32, W: (4096, 4096), A: (16, 4096), B: (4096, 16)

Strategy: flatten x to (16384, 4096) rows, shard rows across 8 cores
(2048 rows each), replicate W/A/B. All matmul operands in bf16 (PSUM
accumulation stays fp32; end-to-end rel err ~2e-3 vs the 2e-2 gate).

Per core, single x-resident block (2048 rows = 128 KB/partition bf16):
  - x loads m-split into 512-col quarters so mt-quarter q + stage-A
    chunk h=q unlock after 1/4 of x lands (keeps PE fed during the
    HBM-bound load).
  - stage A (tT = (x @ B).T) 4-way column-tiled: k-tile k -> PE col
    group k%4, partials in 4 PSUM banks at partition stripes
    {32j..32j+15}; fused per-k with main quarter q=g for oc0 so the
    in-order PE queue never head-of-line blocks on x arrival.
  - LoRA fold for oc0: one K=128 matmul per mt against a2rep (2*A at
    the 4 stripes, zeros elsewhere; tT4 memset kills NaN garbage).
  - LoRA fold for oc1..7: partial stripes packed+duplicated into tTp
    rows 0-63 / 64-127 via SBUF->SBUF DMAs; accum is 2 concurrent
    row-tiled K=64 matmuls per quarter (tile_position (0,0)/(64,0)).
  - main GEMM: per o-chunk, 4 mt-quarters of 4 PSUM banks; x-tile
    stationary, W streamed as k-quads [128, 4*512] on sync.
Queues: W/consts/dups on sync, out stores alternate scalar/sync,
x loads alternate gpsimd/scalar.
"""

import sys

if "/opt/trn_rl_repo" not in sys.path:
    sys.path.insert(0, "/opt/trn_rl_repo")

import numpy as np
import ml_dtypes

import concourse.bass as bass
import concourse.mybir as mybir
import concourse.tile as tile

N_CORES = 8
D = 4096
RANK = 16
ROWS_TOTAL = 4 * 4096          # 16384
M = ROWS_TOTAL // N_CORES      # 2048 rows per core
P = 128
KT = D // P                    # 32 k-tiles
OC = 512                       # o-chunk width (one PSUM bank)
N_OC = D // OC                 # 8
MT = M // P                    # 16 m-tiles
MQ = 4                         # m-tiles per quarter (PSUM banks)
NQ = MT // MQ                  # 4 quarters
KQ = 4                         # k-tiles per W quad DMA
N_KQ = KT // KQ                # 8

F32 = mybir.dt.float32
BF16 = mybir.dt.bfloat16
BF16_NP = ml_dtypes.bfloat16

N_WARMUP = 8


def split_wide_waits(nc, max_waits=1):
    """walrus in this container rejects >1 sync wait per instruction;
    move excess waits onto preceding same-engine NoOps."""
    n_split = 0
    for f in nc.m.functions:
        for bb in f.blocks:
            new_insts = []
            for inst in bb.instructions:
                si = getattr(inst, "sync_info", None)
                if si is not None and si.on_wait and len(si.on_wait) > max_waits:
                    waits = list(si.on_wait)
                    keep = waits[-max_waits:]
                    extra = waits[:-max_waits]
                    for i in range(0, len(extra), max_waits):
                        chunk = extra[i:i + max_waits]
                        nop = mybir.InstNoOp(
                            name=f"{inst.name}_wsplit{i}",
                            sync_info=mybir.SyncInfo(on_wait=chunk, on_update=[]),
                            bass_nofuse=True,
                            engine=inst.engine,
                        )
                        new_insts.append(nop)
                        n_split += 1
                    si.on_wait = keep
                new_insts.append(inst)
            bb.instructions[:] = new_insts
    return n_split


def build_program():
    nc = bass.Bass()
    xt = nc.declare_dram_parameter("xt", [D, M], BF16, isOutput=False)
    wt = nc.declare_dram_parameter("wt", [D, D], BF16, isOutput=False)
    # bmat pre-arranged on host: [128, KT*RANK], col-block k = rows k*128..+128
    bmat = nc.declare_dram_parameter("bmat", [P, KT * RANK], BF16, isOutput=False)
    # a2rep: 2*A at partition stripes {32j..32j+15}, zeros elsewhere (K=128 fold)
    a2rep = nc.declare_dram_parameter("a2rep", [P, D], BF16, isOutput=False)
    # a2p: 2*A tiled 4x in rows 0-63 and again in rows 64-127 (K=64 pair fold)
    a2p = nc.declare_dram_parameter("a2p", [P, D], BF16, isOutput=False)
    out = nc.declare_dram_parameter("out", [M, D], F32, isOutput=True)

    with tile.TileContext(nc) as tc:
        with (
            tc.tile_pool(name="xpool", bufs=1) as xpool,
            tc.tile_pool(name="wpool", bufs=9) as wpool,
            tc.tile_pool(name="opool", bufs=2) as opool,
            tc.tile_pool(name="cpool", bufs=1) as cpool,
            tc.tile_pool(name="tpool", bufs=1) as tpool,
            tc.tile_pool(name="ppool", bufs=8, space="PSUM") as ppool,
        ):
            # constants on sync queue
            btile = cpool.tile([P, KT * RANK], BF16, tag="bt")
            nc.sync.dma_start(btile[:], bmat[:])
            atile_r = cpool.tile([P, D], BF16, tag="ar")
            nc.sync.dma_start(atile_r[:], a2rep[:])
            atile_p = cpool.tile([P, D], BF16, tag="ap")
            nc.sync.dma_start(atile_p[:], a2p[:])

            # x fully resident: col block k holds x.T[k*128:(k+1)*128, :]
            xall = xpool.tile([P, KT * M], BF16, tag="x")

            def xsl(k, c0, cw):
                return xall[:, k * M + c0: k * M + c0 + cw]

            # m-split x loads: m-quarter mq unlocks mt-quarter mq and
            # stage-A chunk h=mq after only 1/4 of x has landed.
            nx = 0
            for mq in range(NQ):
                for k in range(KT):
                    eng = nc.gpsimd if nx % 2 == 0 else nc.scalar
                    nx += 1
                    eng.dma_start(
                        xsl(k, mq * OC, OC),
                        xt[k * P:(k + 1) * P, mq * OC:(mq + 1) * OC],
                    )

            # tT4 [128, M]: stage-A partials at stripes {32j..32j+15};
            # memset clears garbage stripes (read by the oc0 K=128 fold).
            tT4 = tpool.tile([P, M], BF16, tag="tT")
            nc.vector.memset(tT4[:], 0.0)
            # tTp [128, M]: stripes packed to rows {16j..16j+15} and
            # duplicated at rows 64+, for the row-tiled K=64 pair fold.
            tTp = tpool.tile([P, M], BF16, tag="tP")

            # HAM warmup: dummy matmuls so the PE clock ramps to 8/8.
            junk = ppool.tile([P, OC], F32, tag="acc", name="junk")
            for i in range(N_WARMUP):
                nc.tensor.matmul(
                    junk[:],
                    btile[:, :P],
                    btile[:, :OC],
                    start=(i == 0),
                    stop=(i == N_WARMUP - 1),
                )

            def w_load(oc):
                wtiles = []
                for k4 in range(N_KQ):
                    wtile = wpool.tile([P, KQ * OC], BF16, tag="wt")
                    nc.sync.dma_start(
                        wtile.rearrange("p (b c) -> p b c", b=KQ),
                        wt[k4 * KQ * P:(k4 + 1) * KQ * P,
                           oc * OC:(oc + 1) * OC].rearrange(
                               "(b p) c -> p b c", p=P),
                    )
                    wtiles.append(wtile)
                return wtiles

            def finish_quarter(oc, q, psq, packed):
                ot = opool.tile([P, MQ * OC], F32, tag="ot")
                if packed:
                    # row-tiled K=64 pairs: even mi rows 0-63, odd 64-127
                    for mi in range(MQ):
                        mt = q * MQ + mi
                        r0 = 0 if mi % 2 == 0 else 64
                        nc.tensor.matmul(
                            psq[mi][:],
                            tTp[r0:r0 + 64, mt * P:(mt + 1) * P],
                            atile_p[r0:r0 + 64, oc * OC:(oc + 1) * OC],
                            start=False,
                            stop=True,
                            tile_position=(r0, 0),
                        )
                else:
                    for mi in range(MQ):
                        mt = q * MQ + mi
                        nc.tensor.matmul(
                            psq[mi][:],
                            tT4[:, mt * P:(mt + 1) * P],
                            atile_r[:, oc * OC:(oc + 1) * OC],
                            start=False,
                            stop=True,
                        )
                for mi in range(MQ):
                    nc.vector.tensor_copy(
                        ot[:, mi * OC:(mi + 1) * OC], psq[mi][:]
                    )
                seng = nc.scalar if (oc * NQ + q) % 2 == 0 else nc.sync
                seng.dma_start(
                    out[q * MQ * P:(q + 1) * MQ * P,
                        oc * OC:(oc + 1) * OC].rearrange(
                            "(b p) c -> p b c", p=P),
                    ot.rearrange("p (b c) -> p b c", b=MQ),
                )

            # oc0 fused with stage A: per k, 1 stage-A MM + 4 main MMs,
            # all gated on the same arriving x tile (k, mq=g).
            wtiles = w_load(0)
            for g in range(NQ):
                pa = [
                    ppool.tile([P, OC], F32, tag="acc", name=f"pa_{g}_{j}")
                    for j in range(4)
                ]
                psq = [
                    ppool.tile([P, OC], F32, tag="acc", name=f"ps_0_{g}_{mi}")
                    for mi in range(MQ)
                ]
                # k-quad interleave: 16 main MMs first (each gated only
                # on its own arriving x tile — no head-of-line blocking),
                # then the 4 adjacent col-tiled stage-A MMs (4-way
                # overlapped) once all 4 tiles of the quad are present.
                for k4 in range(N_KQ):
                    for kk in range(KQ):
                        k = KQ * k4 + kk
                        for mi in range(MQ):
                            mt = g * MQ + mi
                            nc.tensor.matmul(
                                psq[mi][:],
                                xsl(k, mt * P, P),
                                wtiles[k4][:, kk * OC:(kk + 1) * OC],
                                start=(k == 0),
                                stop=False,
                            )
                    for kk in range(KQ):
                        k = KQ * k4 + kk
                        j = k % 4
                        nc.tensor.matmul(
                            pa[j][32 * j:32 * j + RANK, :],
                            btile[:, k * RANK:(k + 1) * RANK],
                            xsl(k, g * OC, OC),
                            start=(k < 4),
                            stop=(k >= KT - 4),
                            tile_position=(0, 32 * j),
                        )
                for j in range(4):
                    nc.vector.tensor_copy(
                        tT4[32 * j:32 * j + RANK, g * OC:(g + 1) * OC],
                        pa[j][32 * j:32 * j + RANK, :],
                    )
                # pack + duplicate stripes into tTp (SBUF->SBUF, sync queue)
                for j in range(4):
                    sstr = tT4[32 * j:32 * j + RANK, g * OC:(g + 1) * OC]
                    nc.gpsimd.dma_start(
                        tTp[16 * j:16 * j + RANK, g * OC:(g + 1) * OC], sstr)
                    nc.gpsimd.dma_start(
                        tTp[64 + 16 * j:64 + 16 * j + RANK,
                            g * OC:(g + 1) * OC], sstr)
                finish_quarter(0, g, psq, packed=False)

            for oc in range(1, N_OC):
                wtiles = w_load(oc)
                for q in range(NQ):
                    psq = [
                        ppool.tile([P, OC], F32, tag="acc",
                                   name=f"ps_{oc}_{q}_{mi}")
                        for mi in range(MQ)
                    ]
                    for k4 in range(N_KQ):
                        for kk in range(KQ):
                            k = KQ * k4 + kk
                            for mi in range(MQ):
                                mt = q * MQ + mi
                                nc.tensor.matmul(
                                    psq[mi][:],
                                    xsl(k, mt * P, P),
                                    wtiles[k4][:, kk * OC:(kk + 1) * OC],
                                    start=(k == 0),
                                    stop=False,
                                )
                    finish_quarter(oc, q, psq, packed=True)

    split_wide_waits(nc)
    return nc


_NC_CACHE = [None]


def kernel(x, weight, lora_A, lora_B):
    from concourse.bass_utils import run_bass_kernel_spmd

    x = np.asarray(x, dtype=np.float32)
    weight = np.asarray(weight, dtype=np.float32)
    lora_A = np.asarray(lora_A, dtype=np.float32)
    lora_B = np.asarray(lora_B, dtype=np.float32)

    x2 = x.reshape(ROWS_TOTAL, D)
    wt = np.ascontiguousarray(weight.T).astype(BF16_NP)
    a2 = (2.0 * lora_A).astype(BF16_NP)
    # a2rep: 2*A at stripes {32j..32j+15}, zeros elsewhere
    a2rep = np.zeros((P, D), dtype=BF16_NP)
    for j in range(4):
        a2rep[32 * j:32 * j + RANK, :] = a2
    # a2p: 2*A tiled at rows {16j..16j+15} and duplicated at rows 64+
    a2p = np.zeros((P, D), dtype=BF16_NP)
    for j in range(4):
        a2p[16 * j:16 * j + RANK, :] = a2
        a2p[64 + 16 * j:64 + 16 * j + RANK, :] = a2
    # pre-arrange B: [128, KT*RANK], col-block k holds rows k*128..(k+1)*128
    bmat = np.ascontiguousarray(
        lora_B.reshape(KT, P, RANK).transpose(1, 0, 2).reshape(P, KT * RANK)
    ).astype(BF16_NP)

    in_maps = []
    for c in range(N_CORES):
        xt_c = np.ascontiguousarray(
            x2[c * M:(c + 1) * M].T
        ).astype(BF16_NP)
        in_maps.append({"xt": xt_c, "wt": wt, "bmat": bmat,
                        "a2rep": a2rep, "a2p": a2p})

    if _NC_CACHE[0] is None:
        _NC_CACHE[0] = build_program()
    nc = _NC_CACHE[0]

    res = run_bass_kernel_spmd(nc, in_maps, list(range(N_CORES)))
    out = np.concatenate(
        [res.results[c]["out"] for c in range(N_CORES)], axis=0
    )
    return out.reshape(x.shape)


# revision 12
# speedup vs baseline: 1.0156x; 1.0119x over previous
"""LoRA layer kernel for Trainium2, 8-core data-parallel.

out = x @ W.T + 2.0 * ((x @ B) @ A)
  x: (4, 4096, 4096) f32, W: (4096, 4096), A: (16, 4096), B: (4096, 16)

Strategy: flatten x to (16384, 4096) rows, shard rows across 8 cores
(2048 rows each), replicate W/A/B. All matmul operands in bf16 (PSUM
accumulation stays fp32; end-to-end rel err ~2e-3 vs the 2e-2 gate).

Per core, single x-resident block (2048 rows = 128 KB/partition bf16):
  - x loads m-split into 512-col quarters so mt-quarter q + stage-A
    chunk h=q unlock after 1/4 of x lands (keeps PE fed during the
    HBM-bound load).
  - stage A (tT = (x @ B).T) 4-way column-tiled: k-tile k -> PE col
    group k%4, partials in 4 PSUM banks at partition stripes
    {32j..32j+15}; fused per-k with main quarter q=g for oc0 so the
    in-order PE queue never head-of-line blocks on x arrival.
  - LoRA fold for oc0: one K=128 matmul per mt against a2rep (2*A at
    the 4 stripes, zeros elsewhere; tT4 memset kills NaN garbage).
  - LoRA fold for oc1..7: partial stripes packed+duplicated into tTp
    rows 0-63 / 64-127 via SBUF->SBUF DMAs; accum is 2 concurrent
    row-tiled K=64 matmuls per quarter (tile_position (0,0)/(64,0)).
  - main GEMM: per o-chunk, 4 mt-quarters of 4 PSUM banks; x-tile
    stationary, W streamed as k-quads [128, 4*512] on sync.
Queues: W/consts/dups on sync, out stores alternate scalar/sync,
x loads alternate gpsimd/scalar.
"""

import sys

if "/opt/trn_rl_repo" not in sys.path:
    sys.path.insert(0, "/opt/trn_rl_repo")

import numpy as np
import ml_dtypes

import concourse.bass as bass
import concourse.mybir as mybir
import concourse.tile as tile

N_CORES = 8
D = 4096
RANK = 16
ROWS_TOTAL = 4 * 4096          # 16384
M = ROWS_TOTAL // N_CORES      # 2048 rows per core
P = 128
KT = D // P                    # 32 k-tiles
OC = 512                       # o-chunk width (one PSUM bank)
N_OC = D // OC                 # 8
MT = M // P                    # 16 m-tiles
MQ = 4                         # m-tiles per quarter (PSUM banks)
NQ = MT // MQ                  # 4 quarters
KQ = 4                         # k-tiles per W quad DMA
N_KQ = KT // KQ                # 8

F32 = mybir.dt.float32
BF16 = mybir.dt.bfloat16
BF16_NP = ml_dtypes.bfloat16

N_WARMUP = 8


def split_wide_waits(nc, max_waits=1):
    """walrus in this container rejects >1 sync wait per instruction;
    move excess waits onto preceding same-engine NoOps."""
    n_split = 0
    for f in nc.m.functions:
        for bb in f.blocks:
            new_insts = []
            for inst in bb.instructions:
                si = getattr(inst, "sync_info", None)
                if si is not None and si.on_wait and len(si.on_wait) > max_waits:
                    waits = list(si.on_wait)
                    keep = waits[-max_waits:]
                    extra = waits[:-max_waits]
                    for i in range(0, len(extra), max_waits):
                        chunk = extra[i:i + max_waits]
                        nop = mybir.InstNoOp(
                            name=f"{inst.name}_wsplit{i}",
                            sync_info=mybir.SyncInfo(on_wait=chunk, on_update=[]),
                            bass_nofuse=True,
                            engine=inst.engine,
                        )
                        new_insts.append(nop)
                        n_split += 1
                    si.on_wait = keep
                new_insts.append(inst)
            bb.instructions[:] = new_insts
    return n_split


def build_program():
    nc = bass.Bass()
    xt = nc.declare_dram_parameter("xt", [D, M], BF16, isOutput=False)
    wt = nc.declare_dram_parameter("wt", [D, D], BF16, isOutput=False)
    # bmat pre-arranged on host: [128, KT*RANK], col-block k = rows k*128..+128
    bmat = nc.declare_dram_parameter("bmat", [P, KT * RANK], BF16, isOutput=False)
    # a2rep: 2*A at partition stripes {32j..32j+15}, zeros elsewhere (K=128 fold)
    a2rep = nc.declare_dram_parameter("a2rep", [P, D], BF16, isOutput=False)
    # a2p: 2*A tiled 4x in rows 0-63 and again in rows 64-127 (K=64 pair fold)
    a2p = nc.declare_dram_parameter("a2p", [P, D], BF16, isOutput=False)
    out = nc.declare_dram_parameter("out", [M, D], F32, isOutput=True)

    with tile.TileContext(nc) as tc:
        with (
            tc.tile_pool(name="xpool", bufs=1) as xpool,
            tc.tile_pool(name="wpool", bufs=9) as wpool,
            tc.tile_pool(name="opool", bufs=2) as opool,
            tc.tile_pool(name="cpool", bufs=1) as cpool,
            tc.tile_pool(name="tpool", bufs=1) as tpool,
            tc.tile_pool(name="ppool", bufs=8, space="PSUM") as ppool,
        ):
            # constants on sync queue
            btile = cpool.tile([P, KT * RANK], BF16, tag="bt")
            nc.sync.dma_start(btile[:], bmat[:])
            atile_r = cpool.tile([P, D], BF16, tag="ar")
            nc.sync.dma_start(atile_r[:], a2rep[:])
            atile_p = cpool.tile([P, D], BF16, tag="ap")
            nc.sync.dma_start(atile_p[:], a2p[:])

            # x fully resident: col block k holds x.T[k*128:(k+1)*128, :]
            xall = xpool.tile([P, KT * M], BF16, tag="x")

            def xsl(k, c0, cw):
                return xall[:, k * M + c0: k * M + c0 + cw]

            # m-split x loads: m-quarter mq unlocks mt-quarter mq and
            # stage-A chunk h=mq after only 1/4 of x has landed.
            nx = 0
            for mq in range(NQ):
                for k in range(KT):
                    eng = nc.gpsimd if nx % 2 == 0 else nc.scalar
                    nx += 1
                    eng.dma_start(
                        xsl(k, mq * OC, OC),
                        xt[k * P:(k + 1) * P, mq * OC:(mq + 1) * OC],
                    )

            # tT4 [128, M]: stage-A partials at stripes {32j..32j+15};
            # memset clears garbage stripes (read by the oc0 K=128 fold).
            tT4 = tpool.tile([P, M], BF16, tag="tT")
            nc.vector.memset(tT4[:], 0.0)
            # tTp [128, M]: stripes packed to rows {16j..16j+15} and
            # duplicated at rows 64+, for the row-tiled K=64 pair fold.
            tTp = tpool.tile([P, M], BF16, tag="tP")

            # HAM warmup: dummy matmuls so the PE clock ramps to 8/8.
            junk = ppool.tile([P, OC], F32, tag="acc", name="junk")
            for i in range(N_WARMUP):
                nc.tensor.matmul(
                    junk[:],
                    btile[:, :P],
                    btile[:, :OC],
                    start=(i == 0),
                    stop=(i == N_WARMUP - 1),
                )

            def w_load(oc):
                wtiles = []
                for k4 in range(N_KQ):
                    wtile = wpool.tile([P, KQ * OC], BF16, tag="wt")
                    nc.sync.dma_start(
                        wtile.rearrange("p (b c) -> p b c", b=KQ),
                        wt[k4 * KQ * P:(k4 + 1) * KQ * P,
                           oc * OC:(oc + 1) * OC].rearrange(
                               "(b p) c -> p b c", p=P),
                    )
                    wtiles.append(wtile)
                return wtiles

            def finish_quarter(oc, q, psq, packed):
                ot = opool.tile([P, MQ * OC], F32, tag="ot")
                if packed:
                    # row-tiled K=64 pairs: even mi rows 0-63, odd 64-127
                    for mi in range(MQ):
                        mt = q * MQ + mi
                        r0 = 0 if mi % 2 == 0 else 64
                        nc.tensor.matmul(
                            psq[mi][:],
                            tTp[r0:r0 + 64, mt * P:(mt + 1) * P],
                            atile_p[r0:r0 + 64, oc * OC:(oc + 1) * OC],
                            start=False,
                            stop=True,
                            tile_position=(r0, 0),
                        )
                else:
                    for mi in range(MQ):
                        mt = q * MQ + mi
                        nc.tensor.matmul(
                            psq[mi][:],
                            tT4[:, mt * P:(mt + 1) * P],
                            atile_r[:, oc * OC:(oc + 1) * OC],
                            start=False,
                            stop=True,
                        )
                for mi in range(MQ):
                    nc.vector.tensor_copy(
                        ot[:, mi * OC:(mi + 1) * OC], psq[mi][:]
                    )
                seng = nc.scalar if (oc * NQ + q) % 2 == 0 else nc.sync
                seng.dma_start(
                    out[q * MQ * P:(q + 1) * MQ * P,
                        oc * OC:(oc + 1) * OC].rearrange(
                            "(b p) c -> p b c", p=P),
                    ot.rearrange("p (b c) -> p b c", b=MQ),
                )

            # oc0 fused with stage A: per k, 1 stage-A MM + 4 main MMs,
            # all gated on the same arriving x tile (k, mq=g).
            wtiles = w_load(0)
            for g in range(NQ):
                pa = [
                    ppool.tile([P, OC], F32, tag="acc", name=f"pa_{g}_{j}")
                    for j in range(4)
                ]
                psq = [
                    ppool.tile([P, OC], F32, tag="acc", name=f"ps_0_{g}_{mi}")
                    for mi in range(MQ)
                ]
                # k-quad interleave: 4 adjacent col-tiled stage-A MMs
                # (keeps their 4-way overlap), then 16 main MMs — all
                # gated on the same 4 arriving x tiles.
                for k4 in range(N_KQ):
                    for kk in range(KQ):
                        k = KQ * k4 + kk
                        j = k % 4
                        nc.tensor.matmul(
                            pa[j][32 * j:32 * j + RANK, :],
                            btile[:, k * RANK:(k + 1) * RANK],
                            xsl(k, g * OC, OC),
                            start=(k < 4),
                            stop=(k >= KT - 4),
                            tile_position=(0, 32 * j),
                        )
                    for kk in range(KQ):
                        k = KQ * k4 + kk
                        for mi in range(MQ):
                            mt = g * MQ + mi
                            nc.tensor.matmul(
                                psq[mi][:],
                                xsl(k, mt * P, P),
                                wtiles[k4][:, kk * OC:(kk + 1) * OC],
                                start=(k == 0),
                                stop=False,
                            )
                for j in range(4):
                    nc.vector.tensor_copy(
                        tT4[32 * j:32 * j + RANK, g * OC:(g + 1) * OC],
                        pa[j][32 * j:32 * j + RANK, :],
                    )
                # pack + duplicate stripes into tTp (SBUF->SBUF, sync queue)
                for j in range(4):
                    sstr = tT4[32 * j:32 * j + RANK, g * OC:(g + 1) * OC]
                    nc.gpsimd.dma_start(
                        tTp[16 * j:16 * j + RANK, g * OC:(g + 1) * OC], sstr)
                    nc.gpsimd.dma_start(
                        tTp[64 + 16 * j:64 + 16 * j + RANK,
                            g * OC:(g + 1) * OC], sstr)
                finish_quarter(0, g, psq, packed=False)

            for oc in range(1, N_OC):
                wtiles = w_load(oc)
                for q in range(NQ):
                    psq = [
                        ppool.tile([P, OC], F32, tag="acc",
                                   name=f"ps_{oc}_{q}_{mi}")
                        for mi in range(MQ)
                    ]
                    for k4 in range(N_KQ):
                        for kk in range(KQ):
                            k = KQ * k4 + kk
                            for mi in range(MQ):
                                mt = q * MQ + mi
                                nc.tensor.matmul(
                                    psq[mi][:],
                                    xsl(k, mt * P, P),
                                    wtiles[k4][:, kk * OC:(kk + 1) * OC],
                                    start=(k == 0),
                                    stop=False,
                                )
                    finish_quarter(oc, q, psq, packed=True)

    split_wide_waits(nc)
    return nc


_NC_CACHE = [None]


def kernel(x, weight, lora_A, lora_B):
    from concourse.bass_utils import run_bass_kernel_spmd

    x = np.asarray(x, dtype=np.float32)
    weight = np.asarray(weight, dtype=np.float32)
    lora_A = np.asarray(lora_A, dtype=np.float32)
    lora_B = np.asarray(lora_B, dtype=np.float32)

    x2 = x.reshape(ROWS_TOTAL, D)
    wt = np.ascontiguousarray(weight.T).astype(BF16_NP)
    a2 = (2.0 * lora_A).astype(BF16_NP)
    # a2rep: 2*A at stripes {32j..32j+15}, zeros elsewhere
    a2rep = np.zeros((P, D), dtype=BF16_NP)
    for j in range(4):
        a2rep[32 * j:32 * j + RANK, :] = a2
    # a2p: 2*A tiled at rows {16j..16j+15} and duplicated at rows 64+
    a2p = np.zeros((P, D), dtype=BF16_NP)
    for j in range(4):
        a2p[16 * j:16 * j + RANK, :] = a2
        a2p[64 + 16 * j:64 + 16 * j + RANK, :] = a2
    # pre-arrange B: [128, KT*RANK], col-block k holds rows k*128..(k+1)*128
    bmat = np.ascontiguousarray(
        lora_B.reshape(KT, P, RANK).transpose(1, 0, 2).reshape(P, KT * RANK)
    ).astype(BF16_NP)

    in_maps = []
    for c in range(N_CORES):
        xt_c = np.ascontiguousarray(
            x2[c * M:(c + 1) * M].T
        ).astype(BF16_NP)
        in_maps.append({"xt": xt_c, "wt": wt, "bmat": bmat,
                        "a2rep": a2rep, "a2p": a2p})

    if _NC_CACHE[0] is None:
        _NC_CACHE[0] = build_program()
    nc = _NC_CACHE[0]

    res = run_bass_kernel_spmd(nc, in_maps, list(range(N_CORES)))
    out = np.concatenate(
        [res.results[c]["out"] for c in range(N_CORES)], axis=0
    )
    return out.reshape(x.shape)


# revision 13
# speedup vs baseline: 1.0226x; 1.0069x over previous
"""LoRA layer kernel for Trainium2, 8-core data-parallel.

out = x @ W.T + 2.0 * ((x @ B) @ A)
  x: (4, 4096, 4096) f32, W: (4096, 4096), A: (16, 4096), B: (4096, 16)

Strategy: flatten x to (16384, 4096) rows, shard rows across 8 cores
(2048 rows each), replicate W/A/B. All matmul operands in bf16 (PSUM
accumulation stays fp32; end-to-end rel err ~2e-3 vs the 2e-2 gate).

Per core, single x-resident block (2048 rows = 128 KB/partition bf16):
  - x loads m-split into 512-col quarters so mt-quarter q + stage-A
    chunk h=q unlock after 1/4 of x lands (keeps PE fed during the
    HBM-bound load).
  - stage A (tT = (x @ B).T) 4-way column-tiled: k-tile k -> PE col
    group k%4, partials in 4 PSUM banks at partition stripes
    {32j..32j+15}; fused per-k with main quarter q=g for oc0 so the
    in-order PE queue never head-of-line blocks on x arrival.
  - LoRA fold for oc0: one K=128 matmul per mt against a2rep (2*A at
    the 4 stripes, zeros elsewhere; tT4 memset kills NaN garbage).
  - LoRA fold for oc1..7: partial stripes packed+duplicated into tTp
    rows 0-63 / 64-127 via SBUF->SBUF DMAs; accum is 2 concurrent
    row-tiled K=64 matmuls per quarter (tile_position (0,0)/(64,0)).
  - main GEMM: per o-chunk, 4 mt-quarters of 4 PSUM banks; x-tile
    stationary, W streamed as k-quads [128, 4*512] on sync.
Queues: W/consts/dups on sync, out stores alternate scalar/sync,
x loads alternate gpsimd/scalar.
"""

import sys

if "/opt/trn_rl_repo" not in sys.path:
    sys.path.insert(0, "/opt/trn_rl_repo")

import numpy as np
import ml_dtypes

import concourse.bass as bass
import concourse.mybir as mybir
import concourse.tile as tile

N_CORES = 8
D = 4096
RANK = 16
ROWS_TOTAL = 4 * 4096          # 16384
M = ROWS_TOTAL // N_CORES      # 2048 rows per core
P = 128
KT = D // P                    # 32 k-tiles
OC = 512                       # o-chunk width (one PSUM bank)
N_OC = D // OC                 # 8
MT = M // P                    # 16 m-tiles
MQ = 4                         # m-tiles per quarter (PSUM banks)
NQ = MT // MQ                  # 4 quarters
KQ = 4                         # k-tiles per W quad DMA
N_KQ = KT // KQ                # 8

F32 = mybir.dt.float32
BF16 = mybir.dt.bfloat16
BF16_NP = ml_dtypes.bfloat16

N_WARMUP = 8


def split_wide_waits(nc, max_waits=1):
    """walrus in this container rejects >1 sync wait per instruction;
    move excess waits onto preceding same-engine NoOps."""
    n_split = 0
    for f in nc.m.functions:
        for bb in f.blocks:
            new_insts = []
            for inst in bb.instructions:
                si = getattr(inst, "sync_info", None)
                if si is not None and si.on_wait and len(si.on_wait) > max_waits:
                    waits = list(si.on_wait)
                    keep = waits[-max_waits:]
                    extra = waits[:-max_waits]
                    for i in range(0, len(extra), max_waits):
                        chunk = extra[i:i + max_waits]
                        nop = mybir.InstNoOp(
                            name=f"{inst.name}_wsplit{i}",
                            sync_info=mybir.SyncInfo(on_wait=chunk, on_update=[]),
                            bass_nofuse=True,
                            engine=inst.engine,
                        )
                        new_insts.append(nop)
                        n_split += 1
                    si.on_wait = keep
                new_insts.append(inst)
            bb.instructions[:] = new_insts
    return n_split


def build_program():
    nc = bass.Bass()
    xt = nc.declare_dram_parameter("xt", [D, M], BF16, isOutput=False)
    wt = nc.declare_dram_parameter("wt", [D, D], BF16, isOutput=False)
    # bmat pre-arranged on host: [128, KT*RANK], col-block k = rows k*128..+128
    bmat = nc.declare_dram_parameter("bmat", [P, KT * RANK], BF16, isOutput=False)
    # a2rep: 2*A at partition stripes {32j..32j+15}, zeros elsewhere (K=128 fold)
    a2rep = nc.declare_dram_parameter("a2rep", [P, D], BF16, isOutput=False)
    # a2p: 2*A tiled 4x in rows 0-63 and again in rows 64-127 (K=64 pair fold)
    a2p = nc.declare_dram_parameter("a2p", [P, D], BF16, isOutput=False)
    out = nc.declare_dram_parameter("out", [M, D], F32, isOutput=True)

    with tile.TileContext(nc) as tc:
        with (
            tc.tile_pool(name="xpool", bufs=1) as xpool,
            tc.tile_pool(name="wpool", bufs=9) as wpool,
            tc.tile_pool(name="opool", bufs=2) as opool,
            tc.tile_pool(name="cpool", bufs=1) as cpool,
            tc.tile_pool(name="tpool", bufs=1) as tpool,
            tc.tile_pool(name="ppool", bufs=8, space="PSUM") as ppool,
        ):
            # constants on sync queue
            btile = cpool.tile([P, KT * RANK], BF16, tag="bt")
            nc.sync.dma_start(btile[:], bmat[:])
            atile_r = cpool.tile([P, D], BF16, tag="ar")
            nc.sync.dma_start(atile_r[:], a2rep[:])
            atile_p = cpool.tile([P, D], BF16, tag="ap")
            nc.sync.dma_start(atile_p[:], a2p[:])

            # x fully resident, laid out [k4][mq][kk][m] so each
            # k-quad x m-quarter block is flat-contiguous: one 512KB DMA
            # per block (32 total), arriving at exactly the granularity
            # the PE consumes (stage quad + 16 main MMs per block).
            xall = xpool.tile([P, KT * M], BF16, tag="x")

            def xsl(k, c0, cw):
                k4, kk = divmod(k, KQ)
                mq, d = divmod(c0, OC)
                assert d + cw <= OC
                base = ((k4 * NQ + mq) * KQ + kk) * OC + d
                return xall[:, base: base + cw]

            nx = 0
            for mq in range(NQ):
                for k4 in range(N_KQ):
                    eng = nc.gpsimd if nx % 2 == 0 else nc.scalar
                    nx += 1
                    base = (k4 * NQ + mq) * KQ * OC
                    eng.dma_start(
                        xall[:, base: base + KQ * OC].rearrange(
                            "p (b c) -> p b c", b=KQ),
                        xt[k4 * KQ * P:(k4 + 1) * KQ * P,
                           mq * OC:(mq + 1) * OC].rearrange(
                               "(b p) c -> p b c", p=P),
                    )

            # tT4 [128, M]: stage-A partials at stripes {32j..32j+15};
            # memset clears garbage stripes (read by the oc0 K=128 fold).
            tT4 = tpool.tile([P, M], BF16, tag="tT")
            nc.vector.memset(tT4[:], 0.0)
            # tTp [128, M]: stripes packed to rows {16j..16j+15} and
            # duplicated at rows 64+, for the row-tiled K=64 pair fold.
            tTp = tpool.tile([P, M], BF16, tag="tP")

            # HAM warmup: dummy matmuls so the PE clock ramps to 8/8.
            junk = ppool.tile([P, OC], F32, tag="acc", name="junk")
            for i in range(N_WARMUP):
                nc.tensor.matmul(
                    junk[:],
                    btile[:, :P],
                    btile[:, :OC],
                    start=(i == 0),
                    stop=(i == N_WARMUP - 1),
                )

            def w_load(oc):
                wtiles = []
                for k4 in range(N_KQ):
                    wtile = wpool.tile([P, KQ * OC], BF16, tag="wt")
                    nc.sync.dma_start(
                        wtile.rearrange("p (b c) -> p b c", b=KQ),
                        wt[k4 * KQ * P:(k4 + 1) * KQ * P,
                           oc * OC:(oc + 1) * OC].rearrange(
                               "(b p) c -> p b c", p=P),
                    )
                    wtiles.append(wtile)
                return wtiles

            def finish_quarter(oc, q, psq, packed):
                ot = opool.tile([P, MQ * OC], F32, tag="ot")
                if packed:
                    # row-tiled K=64 pairs: even mi rows 0-63, odd 64-127
                    for mi in range(MQ):
                        mt = q * MQ + mi
                        r0 = 0 if mi % 2 == 0 else 64
                        nc.tensor.matmul(
                            psq[mi][:],
                            tTp[r0:r0 + 64, mt * P:(mt + 1) * P],
                            atile_p[r0:r0 + 64, oc * OC:(oc + 1) * OC],
                            start=False,
                            stop=True,
                            tile_position=(r0, 0),
                        )
                else:
                    for mi in range(MQ):
                        mt = q * MQ + mi
                        nc.tensor.matmul(
                            psq[mi][:],
                            tT4[:, mt * P:(mt + 1) * P],
                            atile_r[:, oc * OC:(oc + 1) * OC],
                            start=False,
                            stop=True,
                        )
                for mi in range(MQ):
                    nc.vector.tensor_copy(
                        ot[:, mi * OC:(mi + 1) * OC], psq[mi][:]
                    )
                seng = nc.scalar if (oc * NQ + q) % 2 == 0 else nc.sync
                seng.dma_start(
                    out[q * MQ * P:(q + 1) * MQ * P,
                        oc * OC:(oc + 1) * OC].rearrange(
                            "(b p) c -> p b c", p=P),
                    ot.rearrange("p (b c) -> p b c", b=MQ),
                )

            # oc0 fused with stage A: per k, 1 stage-A MM + 4 main MMs,
            # all gated on the same arriving x tile (k, mq=g).
            wtiles = w_load(0)
            for g in range(NQ):
                pa = [
                    ppool.tile([P, OC], F32, tag="acc", name=f"pa_{g}_{j}")
                    for j in range(4)
                ]
                psq = [
                    ppool.tile([P, OC], F32, tag="acc", name=f"ps_0_{g}_{mi}")
                    for mi in range(MQ)
                ]
                # k-quad interleave: 4 adjacent col-tiled stage-A MMs
                # (keeps their 4-way overlap), then 16 main MMs — all
                # gated on the same 4 arriving x tiles.
                for k4 in range(N_KQ):
                    for kk in range(KQ):
                        k = KQ * k4 + kk
                        j = k % 4
                        nc.tensor.matmul(
                            pa[j][32 * j:32 * j + RANK, :],
                            btile[:, k * RANK:(k + 1) * RANK],
                            xsl(k, g * OC, OC),
                            start=(k < 4),
                            stop=(k >= KT - 4),
                            tile_position=(0, 32 * j),
                        )
                    for kk in range(KQ):
                        k = KQ * k4 + kk
                        for mi in range(MQ):
                            mt = g * MQ + mi
                            nc.tensor.matmul(
                                psq[mi][:],
                                xsl(k, mt * P, P),
                                wtiles[k4][:, kk * OC:(kk + 1) * OC],
                                start=(k == 0),
                                stop=False,
                            )
                for j in range(4):
                    nc.vector.tensor_copy(
                        tT4[32 * j:32 * j + RANK, g * OC:(g + 1) * OC],
                        pa[j][32 * j:32 * j + RANK, :],
                    )
                # pack + duplicate stripes into tTp (SBUF->SBUF, sync queue)
                for j in range(4):
                    sstr = tT4[32 * j:32 * j + RANK, g * OC:(g + 1) * OC]
                    nc.gpsimd.dma_start(
                        tTp[16 * j:16 * j + RANK, g * OC:(g + 1) * OC], sstr)
                    nc.gpsimd.dma_start(
                        tTp[64 + 16 * j:64 + 16 * j + RANK,
                            g * OC:(g + 1) * OC], sstr)
                finish_quarter(0, g, psq, packed=False)

            for oc in range(1, N_OC):
                wtiles = w_load(oc)
                for q in range(NQ):
                    psq = [
                        ppool.tile([P, OC], F32, tag="acc",
                                   name=f"ps_{oc}_{q}_{mi}")
                        for mi in range(MQ)
                    ]
                    for k4 in range(N_KQ):
                        for kk in range(KQ):
                            k = KQ * k4 + kk
                            for mi in range(MQ):
                                mt = q * MQ + mi
                                nc.tensor.matmul(
                                    psq[mi][:],
                                    xsl(k, mt * P, P),
                                    wtiles[k4][:, kk * OC:(kk + 1) * OC],
                                    start=(k == 0),
                                    stop=False,
                                )
                    finish_quarter(oc, q, psq, packed=True)

    split_wide_waits(nc)
    return nc


_NC_CACHE = [None]


def kernel(x, weight, lora_A, lora_B):
    from concourse.bass_utils import run_bass_kernel_spmd

    x = np.asarray(x, dtype=np.float32)
    weight = np.asarray(weight, dtype=np.float32)
    lora_A = np.asarray(lora_A, dtype=np.float32)
    lora_B = np.asarray(lora_B, dtype=np.float32)

    x2 = x.reshape(ROWS_TOTAL, D)
    wt = np.ascontiguousarray(weight.T).astype(BF16_NP)
    a2 = (2.0 * lora_A).astype(BF16_NP)
    # a2rep: 2*A at stripes {32j..32j+15}, zeros elsewhere
    a2rep = np.zeros((P, D), dtype=BF16_NP)
    for j in range(4):
        a2rep[32 * j:32 * j + RANK, :] = a2
    # a2p: 2*A tiled at rows {16j..16j+15} and duplicated at rows 64+
    a2p = np.zeros((P, D), dtype=BF16_NP)
    for j in range(4):
        a2p[16 * j:16 * j + RANK, :] = a2
        a2p[64 + 16 * j:64 + 16 * j + RANK, :] = a2
    # pre-arrange B: [128, KT*RANK], col-block k holds rows k*128..(k+1)*128
    bmat = np.ascontiguousarray(
        lora_B.reshape(KT, P, RANK).transpose(1, 0, 2).reshape(P, KT * RANK)
    ).astype(BF16_NP)

    in_maps = []
    for c in range(N_CORES):
        xt_c = np.ascontiguousarray(
            x2[c * M:(c + 1) * M].T
        ).astype(BF16_NP)
        in_maps.append({"xt": xt_c, "wt": wt, "bmat": bmat,
                        "a2rep": a2rep, "a2p": a2p})

    if _NC_CACHE[0] is None:
        _NC_CACHE[0] = build_program()
    nc = _NC_CACHE[0]

    res = run_bass_kernel_spmd(nc, in_maps, list(range(N_CORES)))
    out = np.concatenate(
        [res.results[c]["out"] for c in range(N_CORES)], axis=0
    )
    return out.reshape(x.shape)
